# revision 5
# baseline (speedup 1.0000x reference)
"""Trainium2 Bass kernel for nn_BiStackedLSTMOne.

Model (per reference):
  forward stack: frames 30..61 (32 steps) -> LSTM(512->256) -> LSTM(256->256)
  reverse stack: frames 63,62,61 (3 steps) -> LSTM(512->256) -> LSTM(256->256)
  out = concat(hF, hR) @ W3.T + b3        # (B, 10)

Distribution: data-parallel over batch. 2048 rows -> 8 NeuronCores x 256.

Device layout: "chunk-major, feature-on-partition". A logical (F, B) tensor
with F = nchunks*128 lives in SBUF as (128, nchunks, B): tile[p,k,b] =
X[k*128+p, b]. Gates are computed transposed — gates'[j, b] — so the hidden
state h is produced directly in the layout the next matmul consumes (rhs with
the contraction dim on partitions). Nothing is ever transposed on device; the
host pre-transposes xs and pre-packs the weights.

Matmuls run in float32r (TF32-like, 11 mantissa bits, full PE rate at moving
dim >= 256). Cell state and elementwise math stay fp32. PSUM accumulation
groups are per gate-block, ordered [recurrent, input] so blocks sharing a
2 KiB PSUM bank form strictly sequential groups.
"""

import os
import sys

sys.path.insert(0, "/opt/trn_rl_repo")
if "/root/.axon_site" not in sys.path:
    sys.path.insert(0, "/root/.axon_site")

import numpy as np

import concourse.bacc as bacc
import concourse.bass as bass
import concourse.mybir as mybir
import concourse.tile as tile
from concourse.bass_utils import run_bass_kernel_spmd

F32 = mybir.dt.float32
F32R = mybir.dt.float32r
AF = mybir.ActivationFunctionType

NCORES = 8
BC = 256          # batch rows per core
TF = 32           # forward steps (frames 30..61)
TR = 3            # reverse steps (frames 63,62,61)
NT = TF + TR      # x time slots shipped to device
HID = 256
NBLK = 8          # 4H / 128 gate blocks
# gate block -> activation: i (0,1) f (2,3) g (4,5) o (6,7)
BLK_FUNC = [AF.Sigmoid, AF.Sigmoid, AF.Sigmoid, AF.Sigmoid,
            AF.Tanh, AF.Tanh, AF.Sigmoid, AF.Sigmoid]

LAST_RESULTS = {"exec_time_ns": None}


def _install_ntff_hook():
    """Recreate the missing antenv.axon_hooks shim so trace=True works."""
    import types

    try:
        import antenv
    except ImportError:
        return
    if "antenv.axon_hooks" in sys.modules:
        return
    mod = types.ModuleType("antenv.axon_hooks")
    mod._hook = None
    mod.set_axon_ntff_profile_hook = lambda h: setattr(mod, "_hook", h)
    mod.get_axon_ntff_profile_hook = lambda: mod._hook
    sys.modules["antenv.axon_hooks"] = mod
    antenv.axon_hooks = mod
    try:
        from trn_agent_boot.trn_boot import _ntff_profile_via_ctypes

        hook = _ntff_profile_via_ctypes("/opt/axon/libaxon_pjrt.so")
        if hook is not None:
            mod.set_axon_ntff_profile_hook(hook)
    except Exception:
        pass


def build_nc():
    nc = bacc.Bacc(None, target_bir_lowering=False, debug=False)

    x_d = nc.declare_dram_parameter("x", [NT, 128, 4, BC], F32, isOutput=False)
    w_d = {}
    for name, kc in [("wih_f0", 4), ("whh_f0", 2), ("wih_f1", 2), ("whh_f1", 2),
                     ("wih_r0", 4), ("whh_r0", 2), ("wih_r1", 2), ("whh_r1", 2)]:
        w_d[name] = nc.declare_dram_parameter(name, [128, kc, NBLK, 128], F32,
                                              isOutput=False)
    b_d = {}
    for lname in ["f0", "f1", "r0", "r1"]:
        b_d[lname] = nc.declare_dram_parameter(f"bias_{lname}", [128, NBLK], F32,
                                               isOutput=False)
    w3_d = nc.declare_dram_parameter("w3", [128, 4, 16], F32, isOutput=False)
    b3_d = nc.declare_dram_parameter("b3", [16, 1], F32, isOutput=False)
    out_d = nc.declare_dram_parameter("out", [16, BC], F32, isOutput=True)

    with tile.TileContext(nc) as tc:
        with (
            tc.tile_pool(name="wpool", bufs=1) as wpool,
            tc.tile_pool(name="wstage", bufs=2) as wstage,
            tc.tile_pool(name="xpool", bufs=3) as xpool,
            tc.tile_pool(name="xstage", bufs=3) as xstage,
            tc.tile_pool(name="pspool", bufs=4, space="PSUM") as pspool,
            tc.tile_pool(name="apool", bufs=4) as apool,
            tc.tile_pool(name="spool", bufs=2) as spool,
            tc.tile_pool(name="hpool", bufs=2) as hpool,
            tc.tile_pool(name="cpool", bufs=1) as cpool,
            tc.tile_pool(name="opool", bufs=1) as opool,
        ):
            # ---- one-time: weights (DMA raw fp32 bytes, round in place) ----
            w = {}
            for name, dram in w_d.items():
                stg = wstage.tile(list(dram.shape), F32, tag="wstage", name=f"stg_{name}")
                nc.sync.dma_start(stg[:], dram.ap())
                t = wpool.tile(list(dram.shape), F32R, tag=name)
                nc.vector.tensor_copy(t[:], stg[:])
                w[name] = t
            bias = {}
            for lname, dram in b_d.items():
                t = wpool.tile([128, NBLK], F32, tag=f"b_{lname}")
                nc.sync.dma_start(t[:], dram.ap())
                bias[lname] = t
            w3s = wstage.tile([128, 4, 16], F32, tag="wstage")
            nc.sync.dma_start(w3s[:], w3_d.ap())
            w3 = wpool.tile([128, 4, 16], F32R, tag="w3")
            nc.vector.tensor_copy(w3[:], w3s[:])
            b3 = wpool.tile([16, 1], F32, tag="b3")
            nc.sync.dma_start(b3[:], b3_d.ap())

            # ---- x streaming ----
            xs = {}

            def load_x(t):
                stg = xstage.tile([128, 4, BC], F32, tag="xs", name=f"xstg{t}")
                nc.sync.dma_start(stg[:], x_d.ap()[t])
                xt = xpool.tile([128, 4, BC], F32R, tag="x", name=f"x{t}")
                nc.vector.tensor_copy(xt[:], stg[:])
                xs[t] = xt

            def lstm_step(lname, x_in, kc_in, first, c_t, h_prev):
                """One LSTM cell step in transposed layout. Returns h (f32r)."""
                wih = w[f"wih_{lname}"]
                whh = w[f"whh_{lname}"]
                bs = bias[lname]
                acts = []
                for half in (0, 1):
                    ps = pspool.tile([128, 4, BC], F32, tag="ps")
                    for mloc in range(4):
                        m = half * 4 + mloc
                        n_in_group = kc_in + (0 if first else 2)
                        gi = 0
                        if not first:
                            for kc in (0, 1):
                                nc.tensor.matmul(
                                    ps[:, mloc, :], whh[:, kc, m, :],
                                    h_prev[:, kc, :],
                                    start=(gi == 0), stop=(gi == n_in_group - 1),
                                )
                                gi += 1
                        for kc in range(kc_in):
                            nc.tensor.matmul(
                                ps[:, mloc, :], wih[:, kc, m, :], x_in[:, kc, :],
                                start=(gi == 0), stop=(gi == n_in_group - 1),
                            )
                            gi += 1
                    a = apool.tile([128, 4, BC], F32, tag="acts")
                    for mloc in range(4):
                        m = half * 4 + mloc
                        nc.scalar.activation(
                            a[:, mloc, :], ps[:, mloc, :], BLK_FUNC[m],
                            bias=bs[:, m:m + 1],
                        )
                    acts.append(a)
                a_ifo, a_go = acts  # blocks [i0 i1 f0 f1], [g0 g1 o0 o2]
                sig_i = a_ifo[:, 0:2, :]
                sig_f = a_ifo[:, 2:4, :]
                tan_g = a_go[:, 0:2, :]
                sig_o = a_go[:, 2:4, :]
                if first:
                    nc.vector.tensor_mul(c_t[:], sig_i, tan_g)
                else:
                    m1 = spool.tile([128, 2, BC], F32, tag="m1")
                    nc.vector.tensor_mul(m1[:], sig_i, tan_g)
                    nc.vector.tensor_mul(c_t[:], sig_f, c_t[:])
                    nc.vector.tensor_add(c_t[:], c_t[:], m1[:])
                tc_ = spool.tile([128, 2, BC], F32, tag="tc")
                nc.scalar.activation(tc_[:], c_t[:], AF.Tanh)
                h = hpool.tile([128, 2, BC], F32R, tag=f"h_{lname}")
                nc.vector.tensor_mul(h[:], sig_o, tc_[:])
                return h

            # ---- forward stack ----
            load_x(0)
            load_x(1)
            c = {ln: cpool.tile([128, 2, BC], F32, tag=f"c_{ln}", name=f"c_{ln}")
                 for ln in ["f0", "f1", "r0", "r1"]}
            h0 = h1 = None
            hF = None
            for t in range(TF):
                if t + 2 < NT:
                    load_x(t + 2)
                h0 = lstm_step("f0", xs[t], 4, t == 0, c["f0"], h0)
                del xs[t]
                h1 = lstm_step("f1", h0, 2, t == 0, c["f1"], h1)
            hF = h1

            # ---- reverse stack (3 steps; x slots TF..TF+2 = frames 63,62,61) --
            h0 = h1 = None
            load_x(NT - 1)
            for r in range(TR):
                h0 = lstm_step("r0", xs[TF + r], 4, r == 0, c["r0"], h0)
                del xs[TF + r]
                h1 = lstm_step("r1", h0, 2, r == 0, c["r1"], h1)
            hR = h1

            # ---- classifier: out[n,b] = sum_k W3[n,k] latent[k,b] + b3 ----
            ps = pspool.tile([128, 4, BC], F32, tag="ps")
            po = ps[:16, 0, :]
            nc.tensor.matmul(po, w3[:, 0, :], hF[:, 0, :], start=True, stop=False)
            nc.tensor.matmul(po, w3[:, 1, :], hF[:, 1, :], start=False, stop=False)
            nc.tensor.matmul(po, w3[:, 2, :], hR[:, 0, :], start=False, stop=False)
            nc.tensor.matmul(po, w3[:, 3, :], hR[:, 1, :], start=False, stop=True)
            ot = opool.tile([16, BC], F32, tag="out")
            nc.scalar.add(ot[:], po, b3[:])
            nc.sync.dma_start(out_d.ap(), ot[:])

    nc.compile()
    return nc


def _pack_weights(Wih, Whh, bih, bhh):
    """Pack into lhsT chunk layout: W.T tiles (128, KC, 8, 128)."""
    fourH, D = Wih.shape
    kc_i, kc_h = D // 128, Whh.shape[1] // 128
    wih = np.ascontiguousarray(
        Wih.reshape(NBLK, 128, kc_i, 128).transpose(3, 2, 0, 1)).astype(np.float32)
    whh = np.ascontiguousarray(
        Whh.reshape(NBLK, 128, kc_h, 128).transpose(3, 2, 0, 1)).astype(np.float32)
    b = np.ascontiguousarray((bih + bhh).reshape(NBLK, 128).T).astype(np.float32)
    return wih, whh, b


_NC_CACHE = {}


def kernel(xs, Wih_f0, Whh_f0, bih_f0, bhh_f0, Wih_f1, Whh_f1, bih_f1, bhh_f1,
           Wih_r0, Whh_r0, bih_r0, bhh_r0, Wih_r1, Whh_r1, bih_r1, bhh_r1,
           W3, b3):
    if os.environ.get("BASS_TRACE"):
        _install_ntff_hook()

    if "nc" not in _NC_CACHE:
        _NC_CACHE["nc"] = build_nc()
    nc = _NC_CACHE["nc"]

    B = xs.shape[0]
    assert B == NCORES * BC

    # frames used: 30..61 forward, then 63,62,61 reversed order
    frames = list(range(62 - TF, 62)) + [63, 62, 61]
    # (B, NT, 512) -> (NT, 512, B)
    xsel = np.ascontiguousarray(
        xs[:, frames, :].transpose(1, 2, 0)).astype(np.float32)

    common = {}
    for lname, (Wih, Whh, bih, bhh) in {
        "f0": (Wih_f0, Whh_f0, bih_f0, bhh_f0),
        "f1": (Wih_f1, Whh_f1, bih_f1, bhh_f1),
        "r0": (Wih_r0, Whh_r0, bih_r0, bhh_r0),
        "r1": (Wih_r1, Whh_r1, bih_r1, bhh_r1),
    }.items():
        wih, whh, b = _pack_weights(np.asarray(Wih), np.asarray(Whh),
                                    np.asarray(bih), np.asarray(bhh))
        common[f"wih_{lname}"] = wih
        common[f"whh_{lname}"] = whh
        common[f"bias_{lname}"] = b

    W3 = np.asarray(W3, dtype=np.float32)          # (10, 512)
    w3p = np.zeros((128, 4, 16), np.float32)
    w3p[:, :, :10] = W3.reshape(10, 4, 128).transpose(2, 1, 0)
    common["w3"] = w3p
    b3p = np.zeros((16, 1), np.float32)
    b3p[:10, 0] = np.asarray(b3, dtype=np.float32)
    common["b3"] = b3p

    in_maps = []
    for core in range(NCORES):
        m = dict(common)
        xc = xsel[:, :, core * BC:(core + 1) * BC].reshape(NT, 4, 128, BC)
        m["x"] = np.ascontiguousarray(xc.transpose(0, 2, 1, 3))
        in_maps.append(m)

    res = run_bass_kernel_spmd(nc, in_maps, list(range(NCORES)))
    LAST_RESULTS["exec_time_ns"] = res.exec_time_ns
    LAST_RESULTS["raw"] = res

    out = np.concatenate(
        [res.results[c]["out"][:10, :].T for c in range(NCORES)], axis=0)
    return np.ascontiguousarray(out.astype(np.float32))


# revision 7
# speedup vs baseline: 1.0954x; 1.0954x over previous
"""Trainium2 Bass kernel for nn_BiStackedLSTMOne.

Model (per reference):
  forward stack: frames 30..61 (32 steps) -> LSTM(512->256) -> LSTM(256->256)
  reverse stack: frames 63,62,61 (3 steps) -> LSTM(512->256) -> LSTM(256->256)
  out = concat(hF, hR) @ W3.T + b3        # (B, 10)

Distribution: data-parallel over batch. 2048 rows -> 8 NeuronCores x 256.

Device layout: "chunk-major, feature-on-partition". A logical (F, B) tensor
with F = nchunks*128 lives in SBUF as (128, nchunks, B): tile[p,k,b] =
X[k*128+p, b]. Gates are computed transposed — gates'[j, b] — so the hidden
state h is produced directly in the layout the next matmul consumes (rhs with
the contraction dim on partitions). Nothing is ever transposed on device; the
host pre-transposes xs and pre-packs the weights.

Matmuls run in float32r (TF32-like, 11 mantissa bits, full PE rate at moving
dim >= 256). Cell state and elementwise math stay fp32. PSUM accumulation
groups are per gate-block, ordered [recurrent, input] so blocks sharing a
2 KiB PSUM bank form strictly sequential groups.
"""

import os
import sys

sys.path.insert(0, "/opt/trn_rl_repo")
if "/root/.axon_site" not in sys.path:
    sys.path.insert(0, "/root/.axon_site")

import numpy as np

import concourse.bacc as bacc
import concourse.bass as bass
import concourse.mybir as mybir
import concourse.tile as tile
from concourse.bass_utils import run_bass_kernel_spmd

F32 = mybir.dt.float32
F32R = mybir.dt.float32r
AF = mybir.ActivationFunctionType

NCORES = 8
BC = 256          # batch rows per core
TF = 32           # forward steps (frames 30..61)
TR = 3            # reverse steps (frames 63,62,61)
NT = TF + TR      # x time slots shipped to device
HID = 256
NBLK = 8          # 4H / 128 gate blocks
# gate blocks after host permutation: f (0,1) i (2,3) g (4,5) o (6,7)
GATE_PERM = [2, 3, 0, 1, 4, 5, 6, 7]   # torch order i,f,g,o -> f,i,g,o
BLK_FUNC = [AF.Sigmoid, AF.Sigmoid, AF.Sigmoid, AF.Sigmoid,
            AF.Tanh, AF.Tanh, AF.Sigmoid, AF.Sigmoid]

LAST_RESULTS = {"exec_time_ns": None}


def _install_ntff_hook():
    """Recreate the missing antenv.axon_hooks shim so trace=True works."""
    import types

    try:
        import antenv
    except ImportError:
        return
    if "antenv.axon_hooks" in sys.modules:
        return
    mod = types.ModuleType("antenv.axon_hooks")
    mod._hook = None
    mod.set_axon_ntff_profile_hook = lambda h: setattr(mod, "_hook", h)
    mod.get_axon_ntff_profile_hook = lambda: mod._hook
    sys.modules["antenv.axon_hooks"] = mod
    antenv.axon_hooks = mod
    try:
        from trn_agent_boot.trn_boot import _ntff_profile_via_ctypes

        hook = _ntff_profile_via_ctypes("/opt/axon/libaxon_pjrt.so")
        if hook is not None:
            mod.set_axon_ntff_profile_hook(hook)
    except Exception:
        pass


def build_nc():
    nc = bacc.Bacc(None, target_bir_lowering=False, debug=False)

    x_d = nc.declare_dram_parameter("x", [NT, 128, 4, BC], F32, isOutput=False)
    w_d = {}
    for name, kc in [("wih_f0", 4), ("whh_f0", 2), ("wih_f1", 2), ("whh_f1", 2),
                     ("wih_r0", 4), ("whh_r0", 2), ("wih_r1", 2), ("whh_r1", 2)]:
        w_d[name] = nc.declare_dram_parameter(name, [128, kc, NBLK, 128], F32,
                                              isOutput=False)
    b_d = {}
    for lname in ["f0", "f1", "r0", "r1"]:
        b_d[lname] = nc.declare_dram_parameter(f"bias_{lname}", [128, NBLK], F32,
                                               isOutput=False)
    w3_d = nc.declare_dram_parameter("w3", [128, 4, 16], F32, isOutput=False)
    b3_d = nc.declare_dram_parameter("b3", [16, 1], F32, isOutput=False)
    out_d = nc.declare_dram_parameter("out", [16, BC], F32, isOutput=True)

    with tile.TileContext(nc) as tc:
        with (
            tc.tile_pool(name="wpool", bufs=1) as wpool,
            tc.tile_pool(name="wstage", bufs=1) as wstage,
            tc.tile_pool(name="xpool", bufs=4) as xpool,
            tc.tile_pool(name="xstage", bufs=3) as xstage,
            tc.tile_pool(name="pspool", bufs=4, space="PSUM") as pspool,
            tc.tile_pool(name="apool", bufs=4) as apool,
            tc.tile_pool(name="spool", bufs=2) as spool,
            tc.tile_pool(name="hpool", bufs=2) as hpool,
            tc.tile_pool(name="cpool", bufs=1) as cpool,
            tc.tile_pool(name="opool", bufs=1) as opool,
        ):
            # ---- one-time: weights (DMA raw fp32 bytes, round in place) ----
            w = {}
            for name, dram in w_d.items():
                stg = wstage.tile(list(dram.shape), F32, tag="wstage", name=f"stg_{name}")
                nc.sync.dma_start(stg[:], dram.ap())
                t = wpool.tile(list(dram.shape), F32R, tag=name)
                nc.vector.tensor_copy(t[:], stg[:])
                w[name] = t
            bias = {}
            for lname, dram in b_d.items():
                t = wpool.tile([128, NBLK], F32, tag=f"b_{lname}")
                nc.sync.dma_start(t[:], dram.ap())
                bias[lname] = t
            w3s = wstage.tile([128, 4, 16], F32, tag="wstage")
            nc.sync.dma_start(w3s[:], w3_d.ap())
            w3 = wpool.tile([128, 4, 16], F32R, tag="w3")
            nc.vector.tensor_copy(w3[:], w3s[:])
            b3 = wpool.tile([16, 1], F32, tag="b3")
            nc.sync.dma_start(b3[:], b3_d.ap())

            # ---- x streaming ----
            xs = {}

            def load_x(t):
                stg = xstage.tile([128, 4, BC], F32, tag="xs", name=f"xstg{t}")
                nc.sync.dma_start(stg[:], x_d.ap()[t])
                xt = xpool.tile([128, 4, BC], F32R, tag="x", name=f"x{t}")
                nc.vector.tensor_copy(xt[:], stg[:])
                xs[t] = xt

            def lstm_step(lname, x_in, kc_in, first, c_t, h_prev):
                """One LSTM cell step in transposed layout. Returns h (f32r)."""
                wih = w[f"wih_{lname}"]
                whh = w[f"whh_{lname}"]
                bs = bias[lname]
                acts = []
                for half in (0, 1):
                    ps = pspool.tile([128, 4, BC], F32, tag="ps")
                    a = apool.tile([128, 4, BC], F32, tag="acts")
                    for mloc in range(4):
                        m = half * 4 + mloc
                        n_in_group = kc_in + (0 if first else 2)
                        gi = 0
                        # input part first: hoistable ahead of h_prev
                        for kc in range(kc_in):
                            nc.tensor.matmul(
                                ps[:, mloc, :], wih[:, kc, m, :], x_in[:, kc, :],
                                start=(gi == 0), stop=(gi == n_in_group - 1),
                            )
                            gi += 1
                        if not first:
                            for kc in (0, 1):
                                nc.tensor.matmul(
                                    ps[:, mloc, :], whh[:, kc, m, :],
                                    h_prev[:, kc, :],
                                    start=(gi == 0), stop=(gi == n_in_group - 1),
                                )
                                gi += 1
                        nc.scalar.activation(
                            a[:, mloc, :], ps[:, mloc, :], BLK_FUNC[m],
                            bias=bs[:, m:m + 1],
                        )
                    acts.append(a)
                a_fi, a_go = acts  # blocks [f0 f1 i0 i1], [g0 g1 o0 o1]
                sig_f = a_fi[:, 0:2, :]
                sig_i = a_fi[:, 2:4, :]
                tan_g = a_go[:, 0:2, :]
                sig_o = a_go[:, 2:4, :]
                if first:
                    nc.vector.tensor_mul(c_t[:], sig_i, tan_g)
                else:
                    m1 = spool.tile([128, 2, BC], F32, tag="m1")
                    nc.vector.tensor_mul(c_t[:], sig_f, c_t[:])
                    nc.vector.tensor_mul(m1[:], sig_i, tan_g)
                    nc.vector.tensor_add(c_t[:], c_t[:], m1[:])
                tc_ = spool.tile([128, 2, BC], F32, tag="tc")
                nc.scalar.activation(tc_[:], c_t[:], AF.Tanh)
                h = hpool.tile([128, 2, BC], F32R, tag=f"h_{lname}")
                nc.vector.tensor_mul(h[:], sig_o, tc_[:])
                return h

            # ---- forward stack, reverse stack interleaved as PE filler ----
            load_x(0)
            load_x(1)
            c = {ln: cpool.tile([128, 2, BC], F32, tag=f"c_{ln}", name=f"c_{ln}")
                 for ln in ["f0", "f1", "r0", "r1"]}
            REV_AT = {8: 0, 16: 1, 24: 2}     # fwd step -> rev super-step
            h0 = h1 = None
            r0 = r1 = None
            for t in range(TF):
                h0 = lstm_step("f0", xs[t], 4, t == 0, c["f0"], h0)
                del xs[t]
                h1 = lstm_step("f1", h0, 2, t == 0, c["f1"], h1)
                if t in REV_AT:
                    r = REV_AT[t]
                    r0 = lstm_step("r0", xs[TF + r], 4, r == 0, c["r0"], r0)
                    del xs[TF + r]
                    r1 = lstm_step("r1", r0, 2, r == 0, c["r1"], r1)
                # prefetch: fwd t+2, plus the rev slot two steps early
                if t + 2 < TF:
                    load_x(t + 2)
                if t + 2 in REV_AT:
                    load_x(TF + REV_AT[t + 2])
            hF = h1
            hR = r1

            # ---- classifier: out[n,b] = sum_k W3[n,k] latent[k,b] + b3 ----
            ps = pspool.tile([128, 4, BC], F32, tag="ps")
            po = ps[:16, 0, :]
            nc.tensor.matmul(po, w3[:, 0, :], hF[:, 0, :], start=True, stop=False)
            nc.tensor.matmul(po, w3[:, 1, :], hF[:, 1, :], start=False, stop=False)
            nc.tensor.matmul(po, w3[:, 2, :], hR[:, 0, :], start=False, stop=False)
            nc.tensor.matmul(po, w3[:, 3, :], hR[:, 1, :], start=False, stop=True)
            ot = opool.tile([16, BC], F32, tag="out")
            nc.scalar.add(ot[:], po, b3[:])
            nc.sync.dma_start(out_d.ap(), ot[:])

    nc.compile()
    return nc


def _pack_weights(Wih, Whh, bih, bhh):
    """Pack into lhsT chunk layout: W.T tiles (128, KC, 8, 128)."""
    fourH, D = Wih.shape
    kc_i, kc_h = D // 128, Whh.shape[1] // 128
    wih = np.ascontiguousarray(
        Wih.reshape(NBLK, 128, kc_i, 128)[GATE_PERM].transpose(3, 2, 0, 1)
    ).astype(np.float32)
    whh = np.ascontiguousarray(
        Whh.reshape(NBLK, 128, kc_h, 128)[GATE_PERM].transpose(3, 2, 0, 1)
    ).astype(np.float32)
    b = np.ascontiguousarray(
        (bih + bhh).reshape(NBLK, 128)[GATE_PERM].T).astype(np.float32)
    return wih, whh, b


_NC_CACHE = {}


def kernel(xs, Wih_f0, Whh_f0, bih_f0, bhh_f0, Wih_f1, Whh_f1, bih_f1, bhh_f1,
           Wih_r0, Whh_r0, bih_r0, bhh_r0, Wih_r1, Whh_r1, bih_r1, bhh_r1,
           W3, b3):
    if os.environ.get("BASS_TRACE"):
        _install_ntff_hook()

    if "nc" not in _NC_CACHE:
        _NC_CACHE["nc"] = build_nc()
    nc = _NC_CACHE["nc"]

    B = xs.shape[0]
    assert B == NCORES * BC

    # frames used: 30..61 forward, then 63,62,61 reversed order
    frames = list(range(62 - TF, 62)) + [63, 62, 61]
    # (B, NT, 512) -> (NT, 512, B)
    xsel = np.ascontiguousarray(
        xs[:, frames, :].transpose(1, 2, 0)).astype(np.float32)

    common = {}
    for lname, (Wih, Whh, bih, bhh) in {
        "f0": (Wih_f0, Whh_f0, bih_f0, bhh_f0),
        "f1": (Wih_f1, Whh_f1, bih_f1, bhh_f1),
        "r0": (Wih_r0, Whh_r0, bih_r0, bhh_r0),
        "r1": (Wih_r1, Whh_r1, bih_r1, bhh_r1),
    }.items():
        wih, whh, b = _pack_weights(np.asarray(Wih), np.asarray(Whh),
                                    np.asarray(bih), np.asarray(bhh))
        common[f"wih_{lname}"] = wih
        common[f"whh_{lname}"] = whh
        common[f"bias_{lname}"] = b

    W3 = np.asarray(W3, dtype=np.float32)          # (10, 512)
    w3p = np.zeros((128, 4, 16), np.float32)
    w3p[:, :, :10] = W3.reshape(10, 4, 128).transpose(2, 1, 0)
    common["w3"] = w3p
    b3p = np.zeros((16, 1), np.float32)
    b3p[:10, 0] = np.asarray(b3, dtype=np.float32)
    common["b3"] = b3p

    in_maps = []
    for core in range(NCORES):
        m = dict(common)
        xc = xsel[:, :, core * BC:(core + 1) * BC].reshape(NT, 4, 128, BC)
        m["x"] = np.ascontiguousarray(xc.transpose(0, 2, 1, 3))
        in_maps.append(m)

    res = run_bass_kernel_spmd(nc, in_maps, list(range(NCORES)))
    LAST_RESULTS["exec_time_ns"] = res.exec_time_ns
    LAST_RESULTS["raw"] = res

    out = np.concatenate(
        [res.results[c]["out"][:10, :].T for c in range(NCORES)], axis=0)
    return np.ascontiguousarray(out.astype(np.float32))


# revision 9
# speedup vs baseline: 1.3193x; 1.2045x over previous
"""Trainium2 Bass kernel for nn_BiStackedLSTMOne.

Model (per reference):
  forward stack: frames 30..61 (32 steps) -> LSTM(512->256) -> LSTM(256->256)
  reverse stack: frames 63,62,61 (3 steps) -> LSTM(512->256) -> LSTM(256->256)
  out = concat(hF, hR) @ W3.T + b3        # (B, 10)

Distribution: data-parallel over batch. 2048 rows -> 8 NeuronCores x 256.

Device layout: "chunk-major, feature-on-partition". A logical (F, B) tensor
with F = nchunks*128 lives in SBUF as (128, nchunks, B): tile[p,k,b] =
X[k*128+p, b]. Gates are computed transposed — gates'[j, b] — so the hidden
state h is produced directly in the layout the next matmul consumes (rhs with
the contraction dim on partitions). Nothing is ever transposed on device; the
host pre-transposes xs and pre-packs the weights.

Matmuls run in float32r (TF32-like, 11 mantissa bits, full PE rate at moving
dim >= 256). Cell state and elementwise math stay fp32. PSUM accumulation
groups are per gate-block, ordered [recurrent, input] so blocks sharing a
2 KiB PSUM bank form strictly sequential groups.
"""

import os
import sys

sys.path.insert(0, "/opt/trn_rl_repo")
if "/root/.axon_site" not in sys.path:
    sys.path.insert(0, "/root/.axon_site")

import numpy as np

import concourse.bacc as bacc
import concourse.bass as bass
import concourse.mybir as mybir
import concourse.tile as tile
from concourse.bass_utils import run_bass_kernel_spmd

F32 = mybir.dt.float32
F32R = mybir.dt.float32r
AF = mybir.ActivationFunctionType

NCORES = 8
BC = 256          # batch rows per core
TF = 32           # forward steps (frames 30..61)
TR = 3            # reverse steps (frames 63,62,61)
NT = TF + TR      # x time slots shipped to device
HID = 256
NBLK = 8          # 4H / 128 gate blocks
# gate blocks after host permutation: f (0,1) i (2,3) g (4,5) o (6,7)
GATE_PERM = [2, 3, 0, 1, 4, 5, 6, 7]   # torch order i,f,g,o -> f,i,g,o
BLK_FUNC = [AF.Sigmoid, AF.Sigmoid, AF.Sigmoid, AF.Sigmoid,
            AF.Tanh, AF.Tanh, AF.Sigmoid, AF.Sigmoid]

LAST_RESULTS = {"exec_time_ns": None}


def _install_ntff_hook():
    """Recreate the missing antenv.axon_hooks shim so trace=True works."""
    import types

    try:
        import antenv
    except ImportError:
        return
    if "antenv.axon_hooks" in sys.modules:
        return
    mod = types.ModuleType("antenv.axon_hooks")
    mod._hook = None
    mod.set_axon_ntff_profile_hook = lambda h: setattr(mod, "_hook", h)
    mod.get_axon_ntff_profile_hook = lambda: mod._hook
    sys.modules["antenv.axon_hooks"] = mod
    antenv.axon_hooks = mod
    try:
        from trn_agent_boot.trn_boot import _ntff_profile_via_ctypes

        hook = _ntff_profile_via_ctypes("/opt/axon/libaxon_pjrt.so")
        if hook is not None:
            mod.set_axon_ntff_profile_hook(hook)
    except Exception:
        pass


def build_nc():
    nc = bacc.Bacc(None, target_bir_lowering=False, debug=False)

    x_d = nc.declare_dram_parameter("x", [NT, 128, 4, BC], F32, isOutput=False)
    w_d = {}
    for name, kc in [("wih_f0", 4), ("whh_f0", 2), ("wih_f1", 2), ("whh_f1", 2),
                     ("wih_r0", 4), ("whh_r0", 2), ("wih_r1", 2),
                     ("whh_r1", 2)]:
        w_d[name] = nc.declare_dram_parameter(name, [128, kc, NBLK, 128], F32,
                                              isOutput=False)
    b_d = {}
    for lname in ["f0", "f1", "r0", "r1"]:
        b_d[lname] = nc.declare_dram_parameter(f"bias_{lname}", [128, NBLK], F32,
                                               isOutput=False)
    w3_d = nc.declare_dram_parameter("w3", [128, 4, 16], F32, isOutput=False)
    b3_d = nc.declare_dram_parameter("b3", [16, 1], F32, isOutput=False)
    out_d = nc.declare_dram_parameter("out", [16, BC], F32, isOutput=True)

    with tile.TileContext(nc) as tc:
        with (
            tc.tile_pool(name="wpool", bufs=1) as wpool,
            tc.tile_pool(name="wstage", bufs=1) as wstage,
            tc.tile_pool(name="xpool", bufs=4) as xpool,
            tc.tile_pool(name="xstage", bufs=3) as xstage,
            tc.tile_pool(name="pspool", bufs=4, space="PSUM") as pspool,
            tc.tile_pool(name="apool", bufs=4) as apool,
            tc.tile_pool(name="spool", bufs=2) as spool,
            tc.tile_pool(name="hpool", bufs=2) as hpool,
            tc.tile_pool(name="cpool", bufs=1) as cpool,
            tc.tile_pool(name="opool", bufs=1) as opool,
        ):
            # ---- x streaming ----
            xs = {}

            def load_x(t):
                stg = xstage.tile([128, 4, BC], F32, tag="xs", name=f"xstg{t}")
                nc.sync.dma_start(stg[:], x_d.ap()[t])
                xt = xpool.tile([128, 4, BC], F32R, tag="x", name=f"x{t}")
                nc.vector.tensor_copy(xt[:], stg[:])
                xs[t] = xt

            # ---- one-time: weights (stage fp32, round to f32r) ----
            w = {}
            bias = {}

            def load_w(name):
                dram = w_d[name]
                stg = wstage.tile(list(dram.shape), F32, tag="wstage",
                                  name=f"stg_{name}")
                nc.sync.dma_start(stg[:], dram.ap())
                t = wpool.tile(list(dram.shape), F32R, tag=name, name=name)
                nc.vector.tensor_copy(t[:], stg[:])
                w[name] = t

            def load_b(lname):
                t = wpool.tile([128, NBLK], F32, tag=f"b_{lname}",
                               name=f"b_{lname}")
                nc.sync.dma_start(t[:], b_d[lname].ap())
                bias[lname] = t

            # f0 weights + first x tiles first so step 0 can start ASAP
            load_w("wih_f0")
            load_w("whh_f0")
            load_b("f0")
            load_x(0)
            load_x(1)
            load_w("wih_f1")
            load_w("whh_f1")
            load_b("f1")
            for name in ["wih_r0", "whh_r0", "wih_r1", "whh_r1"]:
                load_w(name)
            for lname in ["r0", "r1"]:
                load_b(lname)
            w3s = wstage.tile([128, 4, 16], F32, tag="wstage")
            nc.sync.dma_start(w3s[:], w3_d.ap())
            w3 = wpool.tile([128, 4, 16], F32R, tag="w3")
            nc.vector.tensor_copy(w3[:], w3s[:])
            b3 = wpool.tile([16, 1], F32, tag="b3")
            nc.sync.dma_start(b3[:], b3_d.ap())

            def lstm_step(lname, x_in, kc_in, first, c_t, h_prev,
                          rec_first=False):
                """One LSTM cell step in transposed layout. Returns h (f32r)."""
                wih = w[f"wih_{lname}"]
                whh = w[f"whh_{lname}"]
                bs = bias[lname]
                acts = []
                for half in (0, 1):
                    ps = pspool.tile([128, 4, BC], F32, tag="ps")
                    a = apool.tile([128, 4, BC], F32, tag="acts")
                    for mloc in range(4):
                        m = half * 4 + mloc
                        n_in_group = kc_in + (0 if first else 2)
                        gi = 0
                        inp = [(wih, kc, x_in) for kc in range(kc_in)]
                        rec = ([] if first else
                               [(whh, kc, h_prev) for kc in (0, 1)])
                        # L0: input first (hoistable ahead of h_prev).
                        # L1: rec first (h_prev-only dep fills the h0 wait).
                        ops = rec + inp if rec_first else inp + rec
                        for wt, kc, rhs_t in ops:
                            nc.tensor.matmul(
                                ps[:, mloc, :], wt[:, kc, m, :], rhs_t[:, kc, :],
                                start=(gi == 0), stop=(gi == n_in_group - 1),
                            )
                            gi += 1
                        nc.scalar.activation(
                            a[:, mloc, :], ps[:, mloc, :], BLK_FUNC[m],
                            bias=bs[:, m:m + 1],
                        )
                    acts.append(a)
                a_fi, a_go = acts  # blocks [f0 f1 i0 i1], [g0 g1 o0 o1]
                sig_f = a_fi[:, 0:2, :]
                sig_i = a_fi[:, 2:4, :]
                tan_g = a_go[:, 0:2, :]
                sig_o = a_go[:, 2:4, :]
                if first:
                    nc.vector.tensor_mul(c_t[:], sig_i, tan_g)
                else:
                    m1 = spool.tile([128, 2, BC], F32, tag="m1")
                    nc.vector.tensor_mul(c_t[:], sig_f, c_t[:])
                    nc.vector.tensor_mul(m1[:], sig_i, tan_g)
                    nc.vector.tensor_add(c_t[:], c_t[:], m1[:])
                tc_ = spool.tile([128, 2, BC], F32, tag="tc")
                nc.scalar.activation(tc_[:], c_t[:], AF.Tanh)
                h = hpool.tile([128, 2, BC], F32R, tag=f"h_{lname}")
                nc.vector.tensor_mul(h[:], sig_o, tc_[:])
                return h

            # ---- forward stack, reverse stack interleaved as PE filler ----
            c = {ln: cpool.tile([128, 2, BC], F32, tag=f"c_{ln}", name=f"c_{ln}")
                 for ln in ["f0", "f1", "r0", "r1"]}
            REV_AT = {8: 0, 16: 1, 24: 2}     # fwd step -> rev super-step
            h0 = h1 = None
            r0 = r1 = None
            for t in range(TF):
                h0 = lstm_step("f0", xs[t], 4, t == 0, c["f0"], h0)
                del xs[t]
                h1 = lstm_step("f1", h0, 2, t == 0, c["f1"], h1, rec_first=True)
                if t in REV_AT:
                    r = REV_AT[t]
                    r0 = lstm_step("r0", xs[TF + r], 4, r == 0, c["r0"], r0)
                    del xs[TF + r]
                    r1 = lstm_step("r1", r0, 2, r == 0, c["r1"], r1, rec_first=True)
                # prefetch: fwd t+2, plus the rev slot two steps early
                if t + 2 < TF:
                    load_x(t + 2)
                if t + 2 in REV_AT:
                    load_x(TF + REV_AT[t + 2])
            hF = h1
            hR = r1

            # ---- classifier: out[n,b] = sum_k W3[n,k] latent[k,b] + b3 ----
            ps = pspool.tile([128, 4, BC], F32, tag="ps")
            po = ps[:16, 0, :]
            nc.tensor.matmul(po, w3[:, 0, :], hF[:, 0, :], start=True, stop=False)
            nc.tensor.matmul(po, w3[:, 1, :], hF[:, 1, :], start=False, stop=False)
            nc.tensor.matmul(po, w3[:, 2, :], hR[:, 0, :], start=False, stop=False)
            nc.tensor.matmul(po, w3[:, 3, :], hR[:, 1, :], start=False, stop=True)
            ot = opool.tile([16, BC], F32, tag="out")
            nc.scalar.add(ot[:], po, b3[:])
            nc.sync.dma_start(out_d.ap(), ot[:])

    nc.compile()
    return nc


def _pack_weights(Wih, Whh, bih, bhh):
    """Pack into lhsT chunk layout: W.T tiles (128, KC, 8, 128)."""
    fourH, D = Wih.shape
    kc_i, kc_h = D // 128, Whh.shape[1] // 128
    wih = np.ascontiguousarray(
        Wih.reshape(NBLK, 128, kc_i, 128)[GATE_PERM].transpose(3, 2, 0, 1)
    ).astype(np.float32)
    whh = np.ascontiguousarray(
        Whh.reshape(NBLK, 128, kc_h, 128)[GATE_PERM].transpose(3, 2, 0, 1)
    ).astype(np.float32)
    b = np.ascontiguousarray(
        (bih + bhh).reshape(NBLK, 128)[GATE_PERM].T).astype(np.float32)
    return wih, whh, b


_NC_CACHE = {}


def kernel(xs, Wih_f0, Whh_f0, bih_f0, bhh_f0, Wih_f1, Whh_f1, bih_f1, bhh_f1,
           Wih_r0, Whh_r0, bih_r0, bhh_r0, Wih_r1, Whh_r1, bih_r1, bhh_r1,
           W3, b3):
    if os.environ.get("BASS_TRACE"):
        _install_ntff_hook()

    if "nc" not in _NC_CACHE:
        _NC_CACHE["nc"] = build_nc()
    nc = _NC_CACHE["nc"]

    B = xs.shape[0]
    assert B == NCORES * BC

    # frames used: 30..61 forward, then 63,62,61 reversed order
    frames = list(range(62 - TF, 62)) + [63, 62, 61]
    # (B, NT, 512) -> (NT, 512, B)
    xsel = np.ascontiguousarray(
        xs[:, frames, :].transpose(1, 2, 0)).astype(np.float32)

    common = {}
    for lname, (Wih, Whh, bih, bhh) in {
        "f0": (Wih_f0, Whh_f0, bih_f0, bhh_f0),
        "f1": (Wih_f1, Whh_f1, bih_f1, bhh_f1),
        "r0": (Wih_r0, Whh_r0, bih_r0, bhh_r0),
        "r1": (Wih_r1, Whh_r1, bih_r1, bhh_r1),
    }.items():
        wih, whh, b = _pack_weights(np.asarray(Wih), np.asarray(Whh),
                                    np.asarray(bih), np.asarray(bhh))
        common[f"wih_{lname}"] = wih
        common[f"whh_{lname}"] = whh
        common[f"bias_{lname}"] = b

    W3 = np.asarray(W3, dtype=np.float32)          # (10, 512)
    w3p = np.zeros((128, 4, 16), np.float32)
    w3p[:, :, :10] = W3.reshape(10, 4, 128).transpose(2, 1, 0)
    common["w3"] = w3p
    b3p = np.zeros((16, 1), np.float32)
    b3p[:10, 0] = np.asarray(b3, dtype=np.float32)
    common["b3"] = b3p

    in_maps = []
    for core in range(NCORES):
        m = dict(common)
        xc = xsel[:, :, core * BC:(core + 1) * BC].reshape(NT, 4, 128, BC)
        m["x"] = np.ascontiguousarray(xc.transpose(0, 2, 1, 3))
        in_maps.append(m)

    res = run_bass_kernel_spmd(nc, in_maps, list(range(NCORES)))
    LAST_RESULTS["exec_time_ns"] = res.exec_time_ns
    LAST_RESULTS["raw"] = res

    out = np.concatenate(
        [res.results[c]["out"][:10, :].T for c in range(NCORES)], axis=0)
    return np.ascontiguousarray(out.astype(np.float32))


# revision 10
# speedup vs baseline: 1.3815x; 1.0471x over previous
"""Trainium2 Bass kernel for nn_BiStackedLSTMOne.

Model (per reference):
  forward stack: frames 30..61 (32 steps) -> LSTM(512->256) -> LSTM(256->256)
  reverse stack: frames 63,62,61 (3 steps) -> LSTM(512->256) -> LSTM(256->256)
  out = concat(hF, hR) @ W3.T + b3        # (B, 10)

Distribution: data-parallel over batch. 2048 rows -> 8 NeuronCores x 256.

Device layout: "chunk-major, feature-on-partition". A logical (F, B) tensor
with F = nchunks*128 lives in SBUF as (128, nchunks, B): tile[p,k,b] =
X[k*128+p, b]. Gates are computed transposed — gates'[j, b] — so the hidden
state h is produced directly in the layout the next matmul consumes (rhs with
the contraction dim on partitions). Nothing is ever transposed on device; the
host pre-transposes xs and pre-packs the weights.

Matmuls run in float32r (TF32-like, 11 mantissa bits, full PE rate at moving
dim >= 256). Cell state and elementwise math stay fp32. PSUM accumulation
groups are per gate-block, ordered [recurrent, input] so blocks sharing a
2 KiB PSUM bank form strictly sequential groups.
"""

import os
import sys

sys.path.insert(0, "/opt/trn_rl_repo")
if "/root/.axon_site" not in sys.path:
    sys.path.insert(0, "/root/.axon_site")

import numpy as np

import concourse.bacc as bacc
import concourse.bass as bass
import concourse.mybir as mybir
import concourse.tile as tile
from concourse.bass_utils import run_bass_kernel_spmd

F32 = mybir.dt.float32
F32R = mybir.dt.float32r
AF = mybir.ActivationFunctionType

NCORES = 8
BC = 256          # batch rows per core
TF = 32           # forward steps (frames 30..61)
TR = 3            # reverse steps (frames 63,62,61)
NT = TF + TR      # x time slots shipped to device
HID = 256
NBLK = 8          # 4H / 128 gate blocks
# gate blocks after host permutation: f (0,1) i (2,3) g (4,5) o (6,7)
GATE_PERM = [2, 3, 0, 1, 4, 5, 6, 7]   # torch order i,f,g,o -> f,i,g,o
BLK_FUNC = [AF.Sigmoid, AF.Sigmoid, AF.Sigmoid, AF.Sigmoid,
            AF.Tanh, AF.Tanh, AF.Sigmoid, AF.Sigmoid]

LAST_RESULTS = {"exec_time_ns": None}


def _install_ntff_hook():
    """Recreate the missing antenv.axon_hooks shim so trace=True works."""
    import types

    try:
        import antenv
    except ImportError:
        return
    if "antenv.axon_hooks" in sys.modules:
        return
    mod = types.ModuleType("antenv.axon_hooks")
    mod._hook = None
    mod.set_axon_ntff_profile_hook = lambda h: setattr(mod, "_hook", h)
    mod.get_axon_ntff_profile_hook = lambda: mod._hook
    sys.modules["antenv.axon_hooks"] = mod
    antenv.axon_hooks = mod
    try:
        from trn_agent_boot.trn_boot import _ntff_profile_via_ctypes

        hook = _ntff_profile_via_ctypes("/opt/axon/libaxon_pjrt.so")
        if hook is not None:
            mod.set_axon_ntff_profile_hook(hook)
    except Exception:
        pass


def build_nc():
    nc = bacc.Bacc(None, target_bir_lowering=False, debug=False)

    x_d = nc.declare_dram_parameter("x", [NT, 128, 4, BC], F32, isOutput=False)
    w_d = {}
    for name, kc in [("wih_f0", 4), ("whh_f0", 2), ("wih_f1", 2), ("whh_f1", 2),
                     ("wih_r0", 4), ("whh_r0", 2), ("wih_r1", 2),
                     ("whh_r1", 2)]:
        w_d[name] = nc.declare_dram_parameter(name, [128, kc, NBLK, 128], F32,
                                              isOutput=False)
    b_d = {}
    for lname in ["f0", "f1", "r0", "r1"]:
        b_d[lname] = nc.declare_dram_parameter(f"bias_{lname}", [128, NBLK], F32,
                                               isOutput=False)
    w3_d = nc.declare_dram_parameter("w3", [128, 4, 16], F32, isOutput=False)
    b3_d = nc.declare_dram_parameter("b3", [16, 1], F32, isOutput=False)
    out_d = nc.declare_dram_parameter("out", [16, BC], F32, isOutput=True)

    with tile.TileContext(nc) as tc:
        with (
            tc.tile_pool(name="wpool", bufs=1) as wpool,
            tc.tile_pool(name="wstage", bufs=2) as wstage,
            tc.tile_pool(name="xpool", bufs=4) as xpool,
            tc.tile_pool(name="xstage", bufs=3) as xstage,
            tc.tile_pool(name="pspool", bufs=4, space="PSUM") as pspool,
            tc.tile_pool(name="apool", bufs=4) as apool,
            tc.tile_pool(name="spool", bufs=2) as spool,
            tc.tile_pool(name="hpool", bufs=2) as hpool,
            tc.tile_pool(name="cpool", bufs=1) as cpool,
            tc.tile_pool(name="opool", bufs=1) as opool,
        ):
            # ---- x streaming ----
            xs = {}

            def load_x(t):
                stg = xstage.tile([128, 4, BC], F32, tag="xs", name=f"xstg{t}")
                nc.sync.dma_start(stg[:], x_d.ap()[t])
                xt = xpool.tile([128, 4, BC], F32R, tag="x", name=f"x{t}")
                nc.vector.tensor_copy(xt[:], stg[:])
                xs[t] = xt

            # ---- one-time: weights (stage fp32, round to f32r) ----
            w = {}
            bias = {}

            def load_w(name):
                dram = w_d[name]
                stg = wstage.tile(list(dram.shape), F32, tag="wstage",
                                  name=f"stg_{name}")
                nc.sync.dma_start(stg[:], dram.ap())
                t = wpool.tile(list(dram.shape), F32R, tag=name, name=name)
                nc.vector.tensor_copy(t[:], stg[:])
                w[name] = t

            def load_b(lname):
                t = wpool.tile([128, NBLK], F32, tag=f"b_{lname}",
                               name=f"b_{lname}")
                nc.sync.dma_start(t[:], b_d[lname].ap())
                bias[lname] = t

            # f0 weights + first x tiles first so step 0 can start ASAP
            load_w("wih_f0")
            load_w("whh_f0")
            load_b("f0")
            load_x(0)
            load_x(1)
            load_w("wih_f1")
            load_w("whh_f1")
            load_b("f1")
            for name in ["wih_r0", "whh_r0", "wih_r1", "whh_r1"]:
                load_w(name)
            for lname in ["r0", "r1"]:
                load_b(lname)
            w3s = wstage.tile([128, 4, 16], F32, tag="wstage")
            nc.sync.dma_start(w3s[:], w3_d.ap())
            w3 = wpool.tile([128, 4, 16], F32R, tag="w3")
            nc.vector.tensor_copy(w3[:], w3s[:])
            b3 = wpool.tile([16, 1], F32, tag="b3")
            nc.sync.dma_start(b3[:], b3_d.ap())

            def lstm_step(lname, x_in, kc_in, first, c_t, h_prev,
                          rec_first=False):
                """One LSTM cell step in transposed layout.

                x_in / h_prev: lists of (128, BC) chunk APs (contraction
                chunks). c_t: list of 2 persistent cell-state tiles.
                Returns h as a list of 2 fresh (128, BC) f32r tiles, so the
                chunk-0 consumer unblocks before chunk 1 finishes.
                """
                wih = w[f"wih_{lname}"]
                whh = w[f"whh_{lname}"]
                bs = bias[lname]
                acts = []
                for half in (0, 1):
                    ps = pspool.tile([128, 4, BC], F32, tag="ps")
                    a = apool.tile([128, 4, BC], F32, tag="acts")
                    for mloc in range(4):
                        m = half * 4 + mloc
                        n_in_group = kc_in + (0 if first else 2)
                        gi = 0
                        inp = [(wih, kc, x_in[kc]) for kc in range(kc_in)]
                        rec = ([] if first else
                               [(whh, kc, h_prev[kc]) for kc in (0, 1)])
                        # L0: input first (hoistable ahead of h_prev).
                        # L1: rec first (h_prev-only dep fills the h0 wait).
                        ops = rec + inp if rec_first else inp + rec
                        for wt, kc, rhs_ap in ops:
                            nc.tensor.matmul(
                                ps[:, mloc, :], wt[:, kc, m, :], rhs_ap,
                                start=(gi == 0), stop=(gi == n_in_group - 1),
                            )
                            gi += 1
                        nc.scalar.activation(
                            a[:, mloc, :], ps[:, mloc, :], BLK_FUNC[m],
                            bias=bs[:, m:m + 1],
                        )
                    acts.append(a)
                a_fi, a_go = acts  # blocks [f0 f1 i0 i1], [g0 g1 o0 o1]
                h_out = []
                tcs = []
                if not first:
                    for k in (0, 1):
                        nc.vector.tensor_mul(c_t[k][:], a_fi[:, k, :], c_t[k][:])
                for k in (0, 1):
                    if first:
                        nc.vector.tensor_mul(
                            c_t[k][:], a_fi[:, 2 + k, :], a_go[:, k, :])
                    else:
                        m1 = spool.tile([128, BC], F32, tag=f"m1_{k}",
                                        name=f"m1_{k}")
                        nc.vector.tensor_mul(m1[:], a_fi[:, 2 + k, :],
                                             a_go[:, k, :])
                        nc.vector.tensor_add(c_t[k][:], c_t[k][:], m1[:])
                    tc_ = spool.tile([128, BC], F32, tag=f"tc_{k}",
                                     name=f"tc_{k}")
                    nc.scalar.activation(tc_[:], c_t[k][:], AF.Tanh)
                    tcs.append(tc_)
                    h = hpool.tile([128, BC], F32R, tag=f"h_{lname}_{k}",
                                   name=f"h_{lname}_{k}")
                    nc.vector.tensor_mul(h[:], a_go[:, 2 + k, :], tc_[:])
                    h_out.append(h[:])
                return h_out

            # ---- forward stack, reverse stack interleaved as PE filler ----
            c = {ln: [cpool.tile([128, BC], F32, tag=f"c_{ln}_{k}",
                                 name=f"c_{ln}_{k}") for k in (0, 1)]
                 for ln in ["f0", "f1", "r0", "r1"]}
            R0_AT = {5: 0, 13: 1, 27: 2}      # fwd step -> rev-layer0 step
            R1_AT = {7: 0, 15: 1, 29: 2}      # fwd step -> rev-layer1 step
            h0 = h1 = None
            r0 = r1 = None
            rh = {}
            for t in range(TF):
                xa = [xs[t][:, kc, :] for kc in range(4)]
                h0 = lstm_step("f0", xa, 4, t == 0, c["f0"], h0)
                del xs[t]
                h1 = lstm_step("f1", h0, 2, t == 0, c["f1"], h1, rec_first=True)
                if t in R0_AT:
                    r = R0_AT[t]
                    xr = [xs[TF + r][:, kc, :] for kc in range(4)]
                    r0 = lstm_step("r0", xr, 4, r == 0, c["r0"], r0)
                    del xs[TF + r]
                if t in R1_AT:
                    r = R1_AT[t]
                    r1 = lstm_step("r1", r0, 2, r == 0, c["r1"], r1,
                                   rec_first=True)
                # prefetch: fwd t+2, plus the rev slot two steps early
                if t + 2 < TF:
                    load_x(t + 2)
                if t + 2 in R0_AT:
                    load_x(TF + R0_AT[t + 2])
            hF = h1
            hR = r1

            # ---- classifier: out[n,b] = sum_k W3[n,k] latent[k,b] + b3 ----
            ps = pspool.tile([128, 4, BC], F32, tag="ps")
            po = ps[:16, 0, :]
            nc.tensor.matmul(po, w3[:, 0, :], hF[0], start=True, stop=False)
            nc.tensor.matmul(po, w3[:, 1, :], hF[1], start=False, stop=False)
            nc.tensor.matmul(po, w3[:, 2, :], hR[0], start=False, stop=False)
            nc.tensor.matmul(po, w3[:, 3, :], hR[1], start=False, stop=True)
            ot = opool.tile([16, BC], F32, tag="out")
            nc.scalar.add(ot[:], po, b3[:])
            nc.sync.dma_start(out_d.ap(), ot[:])

    nc.compile()
    return nc


def _pack_weights(Wih, Whh, bih, bhh):
    """Pack into lhsT chunk layout: W.T tiles (128, KC, 8, 128)."""
    fourH, D = Wih.shape
    kc_i, kc_h = D // 128, Whh.shape[1] // 128
    wih = np.ascontiguousarray(
        Wih.reshape(NBLK, 128, kc_i, 128)[GATE_PERM].transpose(3, 2, 0, 1)
    ).astype(np.float32)
    whh = np.ascontiguousarray(
        Whh.reshape(NBLK, 128, kc_h, 128)[GATE_PERM].transpose(3, 2, 0, 1)
    ).astype(np.float32)
    b = np.ascontiguousarray(
        (bih + bhh).reshape(NBLK, 128)[GATE_PERM].T).astype(np.float32)
    return wih, whh, b


_NC_CACHE = {}


def kernel(xs, Wih_f0, Whh_f0, bih_f0, bhh_f0, Wih_f1, Whh_f1, bih_f1, bhh_f1,
           Wih_r0, Whh_r0, bih_r0, bhh_r0, Wih_r1, Whh_r1, bih_r1, bhh_r1,
           W3, b3):
    if os.environ.get("BASS_TRACE"):
        _install_ntff_hook()

    if "nc" not in _NC_CACHE:
        _NC_CACHE["nc"] = build_nc()
    nc = _NC_CACHE["nc"]

    B = xs.shape[0]
    assert B == NCORES * BC

    # frames used: 30..61 forward, then 63,62,61 reversed order
    frames = list(range(62 - TF, 62)) + [63, 62, 61]
    # (B, NT, 512) -> (NT, 512, B)
    xsel = np.ascontiguousarray(
        xs[:, frames, :].transpose(1, 2, 0)).astype(np.float32)

    common = {}
    for lname, (Wih, Whh, bih, bhh) in {
        "f0": (Wih_f0, Whh_f0, bih_f0, bhh_f0),
        "f1": (Wih_f1, Whh_f1, bih_f1, bhh_f1),
        "r0": (Wih_r0, Whh_r0, bih_r0, bhh_r0),
        "r1": (Wih_r1, Whh_r1, bih_r1, bhh_r1),
    }.items():
        wih, whh, b = _pack_weights(np.asarray(Wih), np.asarray(Whh),
                                    np.asarray(bih), np.asarray(bhh))
        common[f"wih_{lname}"] = wih
        common[f"whh_{lname}"] = whh
        common[f"bias_{lname}"] = b

    W3 = np.asarray(W3, dtype=np.float32)          # (10, 512)
    w3p = np.zeros((128, 4, 16), np.float32)
    w3p[:, :, :10] = W3.reshape(10, 4, 128).transpose(2, 1, 0)
    common["w3"] = w3p
    b3p = np.zeros((16, 1), np.float32)
    b3p[:10, 0] = np.asarray(b3, dtype=np.float32)
    common["b3"] = b3p

    in_maps = []
    for core in range(NCORES):
        m = dict(common)
        xc = xsel[:, :, core * BC:(core + 1) * BC].reshape(NT, 4, 128, BC)
        m["x"] = np.ascontiguousarray(xc.transpose(0, 2, 1, 3))
        in_maps.append(m)

    res = run_bass_kernel_spmd(nc, in_maps, list(range(NCORES)))
    LAST_RESULTS["exec_time_ns"] = res.exec_time_ns
    LAST_RESULTS["raw"] = res

    out = np.concatenate(
        [res.results[c]["out"][:10, :].T for c in range(NCORES)], axis=0)
    return np.ascontiguousarray(out.astype(np.float32))


# revision 11
# speedup vs baseline: 1.3886x; 1.0052x over previous
"""Trainium2 Bass kernel for nn_BiStackedLSTMOne.

Model (per reference):
  forward stack: frames 30..61 (32 steps) -> LSTM(512->256) -> LSTM(256->256)
  reverse stack: frames 63,62,61 (3 steps) -> LSTM(512->256) -> LSTM(256->256)
  out = concat(hF, hR) @ W3.T + b3        # (B, 10)

Distribution: data-parallel over batch. 2048 rows -> 8 NeuronCores x 256.

Device layout: "chunk-major, feature-on-partition". A logical (F, B) tensor
with F = nchunks*128 lives in SBUF as (128, nchunks, B): tile[p,k,b] =
X[k*128+p, b]. Gates are computed transposed — gates'[j, b] — so the hidden
state h is produced directly in the layout the next matmul consumes (rhs with
the contraction dim on partitions). Nothing is ever transposed on device; the
host pre-transposes xs and pre-packs the weights.

Matmuls run in float32r (TF32-like, 11 mantissa bits, full PE rate at moving
dim >= 256). Cell state and elementwise math stay fp32. PSUM accumulation
groups are per gate-block, ordered [recurrent, input] so blocks sharing a
2 KiB PSUM bank form strictly sequential groups.
"""

import os
import sys

sys.path.insert(0, "/opt/trn_rl_repo")
if "/root/.axon_site" not in sys.path:
    sys.path.insert(0, "/root/.axon_site")

import numpy as np

import concourse.bacc as bacc
import concourse.bass as bass
import concourse.mybir as mybir
import concourse.tile as tile
from concourse.bass_utils import run_bass_kernel_spmd

F32 = mybir.dt.float32
F32R = mybir.dt.float32r
AF = mybir.ActivationFunctionType

NCORES = 8
BC = 256          # batch rows per core
TF = 32           # forward steps (frames 30..61)
TR = 3            # reverse steps (frames 63,62,61)
NT = TF + TR      # x time slots shipped to device
HID = 256
NBLK = 8          # 4H / 128 gate blocks
# gate blocks after host permutation: f (0,1) i (2,3) g (4,5) o (6,7)
GATE_PERM = [2, 3, 0, 1, 4, 5, 6, 7]   # torch order i,f,g,o -> f,i,g,o
BLK_FUNC = [AF.Sigmoid, AF.Sigmoid, AF.Sigmoid, AF.Sigmoid,
            AF.Tanh, AF.Tanh, AF.Sigmoid, AF.Sigmoid]

LAST_RESULTS = {"exec_time_ns": None}


def _install_ntff_hook():
    """Recreate the missing antenv.axon_hooks shim so trace=True works."""
    import types

    try:
        import antenv
    except ImportError:
        return
    if "antenv.axon_hooks" in sys.modules:
        return
    mod = types.ModuleType("antenv.axon_hooks")
    mod._hook = None
    mod.set_axon_ntff_profile_hook = lambda h: setattr(mod, "_hook", h)
    mod.get_axon_ntff_profile_hook = lambda: mod._hook
    sys.modules["antenv.axon_hooks"] = mod
    antenv.axon_hooks = mod
    try:
        from trn_agent_boot.trn_boot import _ntff_profile_via_ctypes

        hook = _ntff_profile_via_ctypes("/opt/axon/libaxon_pjrt.so")
        if hook is not None:
            mod.set_axon_ntff_profile_hook(hook)
    except Exception:
        pass


def build_nc():
    nc = bacc.Bacc(None, target_bir_lowering=False, debug=False)

    x_d = nc.declare_dram_parameter("x", [NT, 128, 4, BC], F32, isOutput=False)
    w_d = {}
    for name, kc in [("wih_f0", 4), ("whh_f0", 2), ("wih_f1", 2), ("whh_f1", 2),
                     ("wih_r0", 4), ("whh_r0", 2), ("wih_r1", 2),
                     ("whh_r1", 2)]:
        w_d[name] = nc.declare_dram_parameter(name, [128, kc, NBLK, 128], F32,
                                              isOutput=False)
    b_d = {}
    for lname in ["f0", "f1", "r0", "r1"]:
        b_d[lname] = nc.declare_dram_parameter(f"bias_{lname}", [128, NBLK], F32,
                                               isOutput=False)
    w3_d = nc.declare_dram_parameter("w3", [128, 4, 16], F32, isOutput=False)
    b3_d = nc.declare_dram_parameter("b3", [16, 1], F32, isOutput=False)
    out_d = nc.declare_dram_parameter("out", [16, BC], F32, isOutput=True)

    with tile.TileContext(nc) as tc:
        with (
            tc.tile_pool(name="wpool", bufs=1) as wpool,
            tc.tile_pool(name="wstage", bufs=2) as wstage,
            tc.tile_pool(name="xpool", bufs=5) as xpool,
            tc.tile_pool(name="xstage", bufs=3) as xstage,
            tc.tile_pool(name="pspool", bufs=4, space="PSUM") as pspool,
            tc.tile_pool(name="apool", bufs=4) as apool,
            tc.tile_pool(name="spool", bufs=2) as spool,
            tc.tile_pool(name="hpool", bufs=2) as hpool,
            tc.tile_pool(name="cpool", bufs=1) as cpool,
            tc.tile_pool(name="opool", bufs=1) as opool,
        ):
            # preload the sigmoid/tanh ACT table set while DMAs run
            warm = opool.tile([1, 2], F32, tag="warm")
            nc.vector.memset(warm[:], 0.0)
            nc.scalar.activation(warm[:, 0:1], warm[:, 0:1], AF.Sigmoid)

            # ---- x streaming ----
            xs = {}

            def load_x(t):
                stg = xstage.tile([128, 4, BC], F32, tag="xs", name=f"xstg{t}")
                nc.sync.dma_start(stg[:], x_d.ap()[t])
                xt = xpool.tile([128, 4, BC], F32R, tag="x", name=f"x{t}")
                nc.vector.tensor_copy(xt[:], stg[:])
                xs[t] = xt

            # ---- one-time: weights (stage fp32, round to f32r) ----
            w = {}
            bias = {}

            def load_w(name):
                dram = w_d[name]
                shape = list(dram.shape)
                t = wpool.tile(shape, F32R, tag=name, name=name)
                kc = shape[1]
                for h0_, h1_ in ([(0, kc // 2), (kc // 2, kc)] if kc > 2
                                 else [(0, kc)]):
                    stg = wstage.tile([128, h1_ - h0_, NBLK, 128], F32,
                                      tag="wstage", name=f"stg_{name}_{h0_}")
                    nc.sync.dma_start(stg[:], dram.ap()[:, h0_:h1_])
                    nc.vector.tensor_copy(t[:, h0_:h1_], stg[:])
                w[name] = t

            def load_b(lname):
                t = wpool.tile([128, NBLK], F32, tag=f"b_{lname}",
                               name=f"b_{lname}")
                nc.sync.dma_start(t[:], b_d[lname].ap())
                bias[lname] = t

            # f0 weights + first x tiles first so step 0 can start ASAP
            load_w("wih_f0")
            load_b("f0")
            load_x(0)
            load_x(1)
            load_w("wih_f1")
            load_b("f1")
            load_w("whh_f0")
            load_w("whh_f1")
            load_x(2)
            for name in ["wih_r0", "whh_r0", "wih_r1", "whh_r1"]:
                load_w(name)
            for lname in ["r0", "r1"]:
                load_b(lname)
            w3s = wstage.tile([128, 4, 16], F32, tag="wstage")
            nc.sync.dma_start(w3s[:], w3_d.ap())
            w3 = wpool.tile([128, 4, 16], F32R, tag="w3")
            nc.vector.tensor_copy(w3[:], w3s[:])
            b3 = wpool.tile([16, 1], F32, tag="b3")
            nc.sync.dma_start(b3[:], b3_d.ap())

            def lstm_step(lname, x_in, kc_in, first, c_t, h_prev,
                          rec_first=False):
                """One LSTM cell step in transposed layout.

                x_in / h_prev: lists of (128, BC) chunk APs (contraction
                chunks). c_t: list of 2 persistent cell-state tiles.
                Returns h as a list of 2 fresh (128, BC) f32r tiles, so the
                chunk-0 consumer unblocks before chunk 1 finishes.
                """
                wih = w[f"wih_{lname}"]
                whh = w[f"whh_{lname}"]
                bs = bias[lname]
                acts = []
                for half in (0, 1):
                    ps = pspool.tile([128, 4, BC], F32, tag="ps")
                    a = apool.tile([128, 4, BC], F32, tag="acts")
                    for mloc in range(4):
                        m = half * 4 + mloc
                        n_in_group = kc_in + (0 if first else 2)
                        gi = 0
                        inp = [(wih, kc, x_in[kc]) for kc in range(kc_in)]
                        rec = ([] if first else
                               [(whh, kc, h_prev[kc]) for kc in (0, 1)])
                        # L0: input first (hoistable ahead of h_prev).
                        # L1: rec first (h_prev-only dep fills the h0 wait).
                        ops = rec + inp if rec_first else inp + rec
                        for wt, kc, rhs_ap in ops:
                            nc.tensor.matmul(
                                ps[:, mloc, :], wt[:, kc, m, :], rhs_ap,
                                start=(gi == 0), stop=(gi == n_in_group - 1),
                            )
                            gi += 1
                        nc.scalar.activation(
                            a[:, mloc, :], ps[:, mloc, :], BLK_FUNC[m],
                            bias=bs[:, m:m + 1],
                        )
                    acts.append(a)
                a_fi, a_go = acts  # blocks [f0 f1 i0 i1], [g0 g1 o0 o1]
                h_out = []
                tcs = []
                if not first:
                    for k in (0, 1):
                        nc.vector.tensor_mul(c_t[k][:], a_fi[:, k, :], c_t[k][:])
                for k in (0, 1):
                    if first:
                        nc.vector.tensor_mul(
                            c_t[k][:], a_fi[:, 2 + k, :], a_go[:, k, :])
                    else:
                        m1 = spool.tile([128, BC], F32, tag=f"m1_{k}",
                                        name=f"m1_{k}")
                        nc.vector.tensor_mul(m1[:], a_fi[:, 2 + k, :],
                                             a_go[:, k, :])
                        nc.vector.tensor_add(c_t[k][:], c_t[k][:], m1[:])
                    tc_ = spool.tile([128, BC], F32, tag=f"tc_{k}",
                                     name=f"tc_{k}")
                    nc.scalar.activation(tc_[:], c_t[k][:], AF.Tanh)
                    tcs.append(tc_)
                    h = hpool.tile([128, BC], F32R, tag=f"h_{lname}_{k}",
                                   name=f"h_{lname}_{k}")
                    nc.vector.tensor_mul(h[:], a_go[:, 2 + k, :], tc_[:])
                    h_out.append(h[:])
                return h_out

            # ---- forward stack, reverse stack interleaved as PE filler ----
            c = {ln: [cpool.tile([128, BC], F32, tag=f"c_{ln}_{k}",
                                 name=f"c_{ln}_{k}") for k in (0, 1)]
                 for ln in ["f0", "f1", "r0", "r1"]}
            R0_AT = {5: 0, 13: 1, 28: 2}      # fwd step -> rev-layer0 step
            R1_AT = {7: 0, 15: 1, 30: 2}      # fwd step -> rev-layer1 step
            h0 = h1 = None
            r0 = r1 = None
            rh = {}
            for t in range(TF):
                xa = [xs[t][:, kc, :] for kc in range(4)]
                h0 = lstm_step("f0", xa, 4, t == 0, c["f0"], h0)
                del xs[t]
                h1 = lstm_step("f1", h0, 2, t == 0, c["f1"], h1, rec_first=True)
                if t in R0_AT:
                    r = R0_AT[t]
                    xr = [xs[TF + r][:, kc, :] for kc in range(4)]
                    r0 = lstm_step("r0", xr, 4, r == 0, c["r0"], r0)
                    del xs[TF + r]
                if t in R1_AT:
                    r = R1_AT[t]
                    r1 = lstm_step("r1", r0, 2, r == 0, c["r1"], r1,
                                   rec_first=True)
                # prefetch: fwd t+3, plus the rev slot two steps early
                if t + 3 < TF:
                    load_x(t + 3)
                if t + 2 in R0_AT:
                    load_x(TF + R0_AT[t + 2])
            hF = h1
            hR = r1

            # ---- classifier: out[n,b] = sum_k W3[n,k] latent[k,b] + b3 ----
            ps = pspool.tile([128, 4, BC], F32, tag="ps")
            po = ps[:16, 0, :]
            nc.tensor.matmul(po, w3[:, 0, :], hF[0], start=True, stop=False)
            nc.tensor.matmul(po, w3[:, 1, :], hF[1], start=False, stop=False)
            nc.tensor.matmul(po, w3[:, 2, :], hR[0], start=False, stop=False)
            nc.tensor.matmul(po, w3[:, 3, :], hR[1], start=False, stop=True)
            ot = opool.tile([16, BC], F32, tag="out")
            nc.scalar.add(ot[:], po, b3[:])
            nc.sync.dma_start(out_d.ap(), ot[:])

    nc.compile()
    return nc


def _pack_weights(Wih, Whh, bih, bhh):
    """Pack into lhsT chunk layout: W.T tiles (128, KC, 8, 128)."""
    fourH, D = Wih.shape
    kc_i, kc_h = D // 128, Whh.shape[1] // 128
    wih = np.ascontiguousarray(
        Wih.reshape(NBLK, 128, kc_i, 128)[GATE_PERM].transpose(3, 2, 0, 1)
    ).astype(np.float32)
    whh = np.ascontiguousarray(
        Whh.reshape(NBLK, 128, kc_h, 128)[GATE_PERM].transpose(3, 2, 0, 1)
    ).astype(np.float32)
    b = np.ascontiguousarray(
        (bih + bhh).reshape(NBLK, 128)[GATE_PERM].T).astype(np.float32)
    return wih, whh, b


_NC_CACHE = {}


def kernel(xs, Wih_f0, Whh_f0, bih_f0, bhh_f0, Wih_f1, Whh_f1, bih_f1, bhh_f1,
           Wih_r0, Whh_r0, bih_r0, bhh_r0, Wih_r1, Whh_r1, bih_r1, bhh_r1,
           W3, b3):
    if os.environ.get("BASS_TRACE"):
        _install_ntff_hook()

    if "nc" not in _NC_CACHE:
        _NC_CACHE["nc"] = build_nc()
    nc = _NC_CACHE["nc"]

    B = xs.shape[0]
    assert B == NCORES * BC

    # frames used: 30..61 forward, then 63,62,61 reversed order
    frames = list(range(62 - TF, 62)) + [63, 62, 61]
    # (B, NT, 512) -> (NT, 512, B)
    xsel = np.ascontiguousarray(
        xs[:, frames, :].transpose(1, 2, 0)).astype(np.float32)

    common = {}
    for lname, (Wih, Whh, bih, bhh) in {
        "f0": (Wih_f0, Whh_f0, bih_f0, bhh_f0),
        "f1": (Wih_f1, Whh_f1, bih_f1, bhh_f1),
        "r0": (Wih_r0, Whh_r0, bih_r0, bhh_r0),
        "r1": (Wih_r1, Whh_r1, bih_r1, bhh_r1),
    }.items():
        wih, whh, b = _pack_weights(np.asarray(Wih), np.asarray(Whh),
                                    np.asarray(bih), np.asarray(bhh))
        common[f"wih_{lname}"] = wih
        common[f"whh_{lname}"] = whh
        common[f"bias_{lname}"] = b

    W3 = np.asarray(W3, dtype=np.float32)          # (10, 512)
    w3p = np.zeros((128, 4, 16), np.float32)
    w3p[:, :, :10] = W3.reshape(10, 4, 128).transpose(2, 1, 0)
    common["w3"] = w3p
    b3p = np.zeros((16, 1), np.float32)
    b3p[:10, 0] = np.asarray(b3, dtype=np.float32)
    common["b3"] = b3p

    in_maps = []
    for core in range(NCORES):
        m = dict(common)
        xc = xsel[:, :, core * BC:(core + 1) * BC].reshape(NT, 4, 128, BC)
        m["x"] = np.ascontiguousarray(xc.transpose(0, 2, 1, 3))
        in_maps.append(m)

    res = run_bass_kernel_spmd(nc, in_maps, list(range(NCORES)))
    LAST_RESULTS["exec_time_ns"] = res.exec_time_ns
    LAST_RESULTS["raw"] = res

    out = np.concatenate(
        [res.results[c]["out"][:10, :].T for c in range(NCORES)], axis=0)
    return np.ascontiguousarray(out.astype(np.float32))
